# revision 10
# baseline (speedup 1.0000x reference)
"""DigitCaps forward kernel for 8 Trainium2 NeuronCores.

Math: the reference collapses to
    s[b, cd] = (1/P) * sum_{p,e} x[b, p, e] * W[0, p, c, d, e]   (cd = c*16+d)
    v = s*|s| / (1 + s^2)                                        (elementwise squash)
    out = v.reshape(BS, C, D, 1)

i.e. one (512, 9216) @ (9216, 160) matmul + tiny elementwise epilogue.

Sharding: 8 cores = 4 batch-groups (128 rows) x 2 output-column halves (80 cols).
Each core reads its x slice (4.72 MB) + its W half (2.95 MB); no collectives.

Device layout: one input tensor per core, K-major, with each 128-deep k-tile
holding [x_tile (128x128) | w_tile (128x80)] side by side. One DMA per chunk
of k-tiles (single sem wait per dependent matmul — TRN2 instructions carry at
most one wait), 72 accumulating fp32 matmuls into one PSUM tile, all-DVE
squash epilogue, one small output DMA.
"""

import numpy as np

BS, P, C, D, E = 512, 1152, 10, 16, 8
K = P * E            # 9216 contraction
CD = C * D           # 160 output cols
KT = 128             # contraction per matmul tile
NKT = K // KT        # 72 k-tiles
NCORES = 8
BG = 4               # batch groups
MB = BS // BG        # 128 rows per group
NH = 2               # cd halves
NHW = CD // NH       # 80 cols per half
COLS = MB + NHW      # 208 cols per k-tile block
ALPHA = 1.0 / P

# DMA chunk sizes in k-tiles, alternating between the two HWDGE rings
# (sync/SP and scalar/ACT) so transfers overlap instead of serializing on one
# ring's per-DMA completion stall. Small first chunk lets the PE start early;
# small last chunk keeps the post-DMA matmul tail short.
CHUNKS = [4, 8, 12, 14, 14, 12, 8]
assert sum(CHUNKS) == NKT
WARMUP_MM = 48       # dummy matmuls to hold PE busy / warm HAM before real work

TRACE = False        # set by test.py to profile
LAST_RESULT = {}     # exec_time_ns etc. for test.py

_CACHED_NC = None


def _build_kernel():
    import concourse.bass as bass
    import concourse.mybir as mybir
    import concourse.tile as tile

    f32 = mybir.dt.float32
    nc = bass.Bass()
    xw_d = nc.dram_tensor("xw", [KT, NKT * COLS], f32, kind="ExternalInput")
    o_d = nc.dram_tensor("o", [MB, NHW], f32, kind="ExternalOutput")

    with tile.TileContext(nc) as tc:
        with (
            tc.tile_pool(name="xwp", bufs=len(CHUNKS)) as xwp,
            tc.tile_pool(name="wu", bufs=1) as wu,
            tc.tile_pool(name="ep", bufs=1) as ep,
            tc.tile_pool(name="pp", bufs=1, space="PSUM") as pp,
            tc.tile_pool(name="pw", bufs=1, space="PSUM") as pw,
        ):
            # --- PE warmup: keep the PE busy (and HAM un-throttled) while the
            # entry preamble and first DMA chunks are in flight.
            warm = wu.tile([KT, 32], f32)
            wps = pw.tile([32, 32], f32)
            nc.vector.memset(warm[:], 0.0)
            for _ in range(WARMUP_MM):
                nc.tensor.matmul(wps[:], warm[:, :32], warm[:], start=True, stop=True)
            # Prewarm ACT tables used by the epilogue.
            wact = wu.tile([1, 1], f32)
            nc.scalar.square(wact[:], warm[:1, :1])
            nc.scalar.add(wact[:], wact[:], 1.0)

            bufs = []
            t0 = 0
            for gi, tpg in enumerate(CHUNKS):
                xwg = xwp.tile([KT, tpg * COLS], f32, tag="xw")
                eng = nc.sync if gi % 2 == 0 else nc.scalar
                eng.dma_start(
                    out=xwg[:], in_=xw_d[:, t0 * COLS:(t0 + tpg) * COLS]
                )
                bufs.append((xwg, t0, tpg))
                t0 += tpg

            ps = pp.tile([MB, NHW], f32)
            for xwg, t0, tpg in bufs:
                for j in range(tpg):
                    t = t0 + j
                    nc.tensor.matmul(
                        ps[:],
                        xwg[:, j * COLS:j * COLS + MB],
                        xwg[:, j * COLS + MB:(j + 1) * COLS],
                        start=(t == 0),
                        stop=(t == NKT - 1),
                    )

            # epilogue: s = ps*ALPHA; v = s*|s| / (1 + s^2)
            # ACT computes q2=(ALPHA*ps)^2 and d2=q2+1 in parallel with DVE's
            # s, -s, |s|, s*|s|; DVE finishes with r=1/d2 and v=n*r.
            s = ep.tile([MB, NHW], f32)
            ng = ep.tile([MB, NHW], f32)
            a = ep.tile([MB, NHW], f32)
            n = ep.tile([MB, NHW], f32)
            q2 = ep.tile([MB, NHW], f32)
            d2 = ep.tile([MB, NHW], f32)
            r = ep.tile([MB, NHW], f32)
            v = ep.tile([MB, NHW], f32)
            nc.scalar.activation(q2[:], ps[:], mybir.ActivationFunctionType.Square,
                                 scale=ALPHA)
            nc.scalar.add(d2[:], q2[:], 1.0)
            nc.vector.tensor_scalar_mul(s[:], ps[:], ALPHA)
            nc.vector.tensor_scalar_mul(ng[:], ps[:], -ALPHA)
            nc.vector.tensor_tensor(a[:], s[:], ng[:], mybir.AluOpType.max)
            nc.vector.tensor_mul(n[:], s[:], a[:])
            nc.vector.reciprocal(r[:], d2[:])
            nc.vector.tensor_mul(v[:], n[:], r[:])
            nc.sync.dma_start(out=o_d[:], in_=v[:])
    _split_multi_waits(nc)
    return nc


def _split_multi_waits(nc):
    """TRN2 instructions carry at most one semaphore wait; walrus rejects
    more. Tile's auto-emitted kernel-tail Drain waits on every engine/DMA
    sem. Split extra waits into standalone single-wait EventSemaphore
    instructions placed just before the owner, on the same engine."""
    import concourse.mybir as mybir

    for f in nc.m.functions:
        for blk in f.blocks:
            out = []
            changed = False
            for inst in blk.instructions:
                si = inst.sync_info
                waits = list(si.on_wait) if si and si.on_wait else []
                if len(waits) > 1:
                    changed = True
                    for k, w in enumerate(waits[:-1]):
                        out.append(mybir.InstEventSemaphore(
                            name=f"{inst.name}-sw{k}",
                            engine=inst.engine,
                            ins=[],
                            outs=[],
                            sync_info=mybir.SyncInfo(on_wait=[w], on_update=[]),
                        ))
                    inst.sync_info = mybir.SyncInfo(
                        on_wait=[waits[-1]],
                        on_update=list(si.on_update) if si.on_update else [],
                    )
                out.append(inst)
            if changed:
                blk.instructions = out


def _prep_inputs(x, W):
    """Build the per-core [k, t, (x|w)] interleaved operand arrays."""
    xr = np.ascontiguousarray(x, dtype=np.float32).reshape(BS, K)
    xgs = []
    for g in range(BG):
        xg = xr[g * MB:(g + 1) * MB, :].T.reshape(NKT, KT, MB)  # (t, k, b)
        xgs.append(np.transpose(xg, (1, 0, 2)))                  # (k, t, b)
    Wf = np.ascontiguousarray(
        np.asarray(W, dtype=np.float32)[0].transpose(0, 3, 1, 2)
    ).reshape(K, CD)
    whs = []
    for h in range(NH):
        wh = Wf[:, h * NHW:(h + 1) * NHW].reshape(NKT, KT, NHW)  # (t, k, n)
        whs.append(np.transpose(wh, (1, 0, 2)))                  # (k, t, n)
    maps = []
    for i in range(NCORES):
        g, h = i % BG, i // BG
        xw = np.concatenate([xgs[g], whs[h]], axis=2)            # (k, t, 208)
        maps.append({"xw": np.ascontiguousarray(xw).reshape(KT, NKT * COLS)})
    return maps


def kernel(x, W):
    global _CACHED_NC, LAST_RESULT
    from concourse.bass_utils import run_bass_kernel_spmd

    x = np.asarray(x, dtype=np.float32)
    W = np.asarray(W, dtype=np.float32)
    assert x.shape == (BS, P, E), x.shape
    assert W.shape == (1, P, C, D, E), W.shape

    if _CACHED_NC is None:
        _CACHED_NC = _build_kernel()
    nc = _CACHED_NC

    in_maps = _prep_inputs(x, W)
    res = run_bass_kernel_spmd(nc, in_maps, core_ids=list(range(NCORES)), trace=TRACE)
    LAST_RESULT = {"exec_time_ns": res.exec_time_ns,
                   "mean_exec_time_ns": res.mean_exec_time_ns,
                   "trace": res.instructions_and_trace}

    out = np.empty((BS, CD), dtype=np.float32)
    for i in range(NCORES):
        g, h = i % BG, i // BG
        out[g * MB:(g + 1) * MB, h * NHW:(h + 1) * NHW] = res.results[i]["o"]
    return out.reshape(BS, C, D, 1)


# revision 12
# speedup vs baseline: 1.2118x; 1.2118x over previous
"""DigitCaps forward kernel for 8 Trainium2 NeuronCores.

Math: the reference collapses to
    s[b, cd] = (1/P) * sum_{p,e} x[b, p, e] * W[0, p, c, d, e]   (cd = c*16+d)
    v = s*|s| / (1 + s^2)                                        (elementwise squash)
    out = v.reshape(BS, C, D, 1)

i.e. one (512, 9216) @ (9216, 160) matmul + tiny elementwise epilogue.

Sharding: 8 cores = 4 batch-groups (128 rows) x 2 output-column halves (80 cols).
Each core reads its x slice (4.72 MB) + its W half (2.95 MB); no collectives.

Device layout: one input tensor per core, K-major, with each 128-deep k-tile
holding [x_tile (128x128) | w_tile (128x80)] side by side. One DMA per chunk
of k-tiles (single sem wait per dependent matmul — TRN2 instructions carry at
most one wait), 72 accumulating fp32 matmuls into one PSUM tile, all-DVE
squash epilogue, one small output DMA.
"""

import numpy as np

BS, P, C, D, E = 512, 1152, 10, 16, 8
K = P * E            # 9216 contraction
CD = C * D           # 160 output cols
KT = 128             # contraction per matmul tile
NKT = K // KT        # 72 k-tiles
NCORES = 8
BG = 4               # batch groups
MB = BS // BG        # 128 rows per group
NH = 2               # cd halves
NHW = CD // NH       # 80 cols per half
COLS = MB + NHW      # 208 cols per k-tile block
ALPHA = 1.0 / P

# DMA chunk sizes in k-tiles, alternating between the two HWDGE rings
# (sync/SP and scalar/ACT) so transfers overlap instead of serializing on one
# ring's per-DMA completion stall. Small first chunk lets the PE start early;
# small last chunk keeps the post-DMA matmul tail short.
CHUNKS = [4, 8, 12, 14, 14, 12, 8]
assert sum(CHUNKS) == NKT
WARMUP_MM = 24       # dummy matmuls to hold PE busy / warm HAM before real work

TRACE = False        # set by test.py to profile
LAST_RESULT = {}     # exec_time_ns etc. for test.py

_CACHED_NC = None


def _build_kernel():
    import concourse.bass as bass
    import concourse.mybir as mybir
    import concourse.tile as tile

    f32 = mybir.dt.float32
    nc = bass.Bass()
    xw_d = nc.dram_tensor("xw", [KT, NKT * COLS], f32, kind="ExternalInput")
    o_d = nc.dram_tensor("o", [NHW, MB], f32, kind="ExternalOutput")

    with tile.TileContext(nc) as tc:
        with (
            tc.tile_pool(name="xwp", bufs=len(CHUNKS)) as xwp,
            tc.tile_pool(name="wu", bufs=1) as wu,
            tc.tile_pool(name="ep", bufs=1) as ep,
            tc.tile_pool(name="pp", bufs=1, space="PSUM") as pp,
            tc.tile_pool(name="pw", bufs=1, space="PSUM") as pw,
        ):
            # --- PE warmup: keep the PE busy (and HAM un-throttled) while the
            # entry preamble and first DMA chunks are in flight.
            warm = wu.tile([KT, 32], f32)
            wps = pw.tile([32, 32], f32)
            nc.vector.memset(warm[:], 0.0)
            for _ in range(WARMUP_MM):
                nc.tensor.matmul(wps[:], warm[:, :32], warm[:], start=True, stop=True)
            # Prewarm ACT tables used by the epilogue.
            wact = wu.tile([1, 1], f32)
            nc.scalar.square(wact[:], warm[:1, :1])
            nc.scalar.add(wact[:], wact[:], 1.0)

            bufs = []
            t0 = 0
            for gi, tpg in enumerate(CHUNKS):
                xwg = xwp.tile([KT, tpg * COLS], f32, tag="xw")
                eng = nc.sync if gi % 2 == 0 else nc.scalar
                eng.dma_start(
                    out=xwg[:], in_=xw_d[:, t0 * COLS:(t0 + tpg) * COLS]
                )
                bufs.append((xwg, t0, tpg))
                t0 += tpg

            # W-half is the stationary operand (80 cols -> cheap LDWEIGHTS,
            # which is the PE bottleneck for fp32: ~2cyc/col at 1.2 GHz);
            # the 128 x columns stream as the moving operand and hide under
            # it. Output comes out transposed: psum[cd, b].
            ps = pp.tile([NHW, MB], f32)
            for xwg, t0, tpg in bufs:
                for j in range(tpg):
                    t = t0 + j
                    nc.tensor.matmul(
                        ps[:],
                        xwg[:, j * COLS + MB:(j + 1) * COLS],
                        xwg[:, j * COLS:j * COLS + MB],
                        start=(t == 0),
                        stop=(t == NKT - 1),
                    )

            # epilogue: s = ps*ALPHA; v = s*|s| / (1 + s^2)
            # ACT computes q2=(ALPHA*ps)^2 and d2=q2+1 in parallel with DVE's
            # s, -s, |s|, s*|s|; DVE finishes with r=1/d2 and v=n*r.
            s = ep.tile([NHW, MB], f32)
            ng = ep.tile([NHW, MB], f32)
            a = ep.tile([NHW, MB], f32)
            n = ep.tile([NHW, MB], f32)
            q2 = ep.tile([NHW, MB], f32)
            d2 = ep.tile([NHW, MB], f32)
            r = ep.tile([NHW, MB], f32)
            v = ep.tile([NHW, MB], f32)
            nc.scalar.activation(q2[:], ps[:], mybir.ActivationFunctionType.Square,
                                 scale=ALPHA)
            nc.scalar.add(d2[:], q2[:], 1.0)
            nc.vector.tensor_scalar_mul(s[:], ps[:], ALPHA)
            nc.vector.tensor_scalar_mul(ng[:], ps[:], -ALPHA)
            nc.vector.tensor_tensor(a[:], s[:], ng[:], mybir.AluOpType.max)
            nc.vector.tensor_mul(n[:], s[:], a[:])
            nc.vector.reciprocal(r[:], d2[:])
            nc.vector.tensor_mul(v[:], n[:], r[:])
            nc.sync.dma_start(out=o_d[:], in_=v[:])
    _split_multi_waits(nc)
    return nc


def _split_multi_waits(nc):
    """TRN2 instructions carry at most one semaphore wait; walrus rejects
    more. Tile's auto-emitted kernel-tail Drain waits on every engine/DMA
    sem. Split extra waits into standalone single-wait EventSemaphore
    instructions placed just before the owner, on the same engine."""
    import concourse.mybir as mybir

    for f in nc.m.functions:
        for blk in f.blocks:
            out = []
            changed = False
            for inst in blk.instructions:
                si = inst.sync_info
                waits = list(si.on_wait) if si and si.on_wait else []
                if len(waits) > 1:
                    changed = True
                    for k, w in enumerate(waits[:-1]):
                        out.append(mybir.InstEventSemaphore(
                            name=f"{inst.name}-sw{k}",
                            engine=inst.engine,
                            ins=[],
                            outs=[],
                            sync_info=mybir.SyncInfo(on_wait=[w], on_update=[]),
                        ))
                    inst.sync_info = mybir.SyncInfo(
                        on_wait=[waits[-1]],
                        on_update=list(si.on_update) if si.on_update else [],
                    )
                out.append(inst)
            if changed:
                blk.instructions = out


def _prep_inputs(x, W):
    """Build the per-core [k, t, (x|w)] interleaved operand arrays."""
    xr = np.ascontiguousarray(x, dtype=np.float32).reshape(BS, K)
    xgs = []
    for g in range(BG):
        xg = xr[g * MB:(g + 1) * MB, :].T.reshape(NKT, KT, MB)  # (t, k, b)
        xgs.append(np.transpose(xg, (1, 0, 2)))                  # (k, t, b)
    Wf = np.ascontiguousarray(
        np.asarray(W, dtype=np.float32)[0].transpose(0, 3, 1, 2)
    ).reshape(K, CD)
    whs = []
    for h in range(NH):
        wh = Wf[:, h * NHW:(h + 1) * NHW].reshape(NKT, KT, NHW)  # (t, k, n)
        whs.append(np.transpose(wh, (1, 0, 2)))                  # (k, t, n)
    maps = []
    for i in range(NCORES):
        g, h = i % BG, i // BG
        xw = np.concatenate([xgs[g], whs[h]], axis=2)            # (k, t, 208)
        maps.append({"xw": np.ascontiguousarray(xw).reshape(KT, NKT * COLS)})
    return maps


def kernel(x, W):
    global _CACHED_NC, LAST_RESULT
    from concourse.bass_utils import run_bass_kernel_spmd

    x = np.asarray(x, dtype=np.float32)
    W = np.asarray(W, dtype=np.float32)
    assert x.shape == (BS, P, E), x.shape
    assert W.shape == (1, P, C, D, E), W.shape

    if _CACHED_NC is None:
        _CACHED_NC = _build_kernel()
    nc = _CACHED_NC

    in_maps = _prep_inputs(x, W)
    res = run_bass_kernel_spmd(nc, in_maps, core_ids=list(range(NCORES)), trace=TRACE)
    LAST_RESULT = {"exec_time_ns": res.exec_time_ns,
                   "mean_exec_time_ns": res.mean_exec_time_ns,
                   "trace": res.instructions_and_trace}

    out = np.empty((BS, CD), dtype=np.float32)
    for i in range(NCORES):
        g, h = i % BG, i // BG
        out[g * MB:(g + 1) * MB, h * NHW:(h + 1) * NHW] = res.results[i]["o"].T
    return out.reshape(BS, C, D, 1)


# revision 13
# speedup vs baseline: 1.3010x; 1.0736x over previous
"""DigitCaps forward kernel for 8 Trainium2 NeuronCores.

Math: the reference collapses to
    s[b, cd] = (1/P) * sum_{p,e} x[b, p, e] * W[0, p, c, d, e]   (cd = c*16+d)
    v = s*|s| / (1 + s^2)                                        (elementwise squash)
    out = v.reshape(BS, C, D, 1)

i.e. one (512, 9216) @ (9216, 160) matmul + tiny elementwise epilogue.

Sharding: 8 cores = 4 batch-groups (128 rows) x 2 output-column halves (80 cols).
Each core reads its x slice (4.72 MB) + its W half (2.95 MB); no collectives.

Device layout: one input tensor per core, K-major, with each 128-deep k-tile
holding [x_tile (128x128) | w_tile (128x80)] side by side. One DMA per chunk
of k-tiles (single sem wait per dependent matmul — TRN2 instructions carry at
most one wait), 72 accumulating fp32 matmuls into one PSUM tile, all-DVE
squash epilogue, one small output DMA.
"""

import numpy as np

BS, P, C, D, E = 512, 1152, 10, 16, 8
K = P * E            # 9216 contraction
CD = C * D           # 160 output cols
KT = 128             # contraction per matmul tile
NKT = K // KT        # 72 k-tiles
NCORES = 8
BG = 4               # batch groups
MB = BS // BG        # 128 rows per group
NH = 2               # cd halves
NHW = CD // NH       # 80 cols per half
COLS = MB + NHW      # 208 cols per k-tile block
ALPHA = 1.0 / P

# DMA chunk sizes in k-tiles, alternating between the two HWDGE rings
# (sync/SP and scalar/ACT) so transfers overlap instead of serializing on one
# ring's per-DMA completion stall. Small first chunk lets the PE start early;
# small last chunk keeps the post-DMA matmul tail short.
CHUNKS = [6, 10, 12, 14, 14, 12, 4]
assert sum(CHUNKS) == NKT
WARMUP_MM = 64       # dummy matmuls to hold PE busy / warm HAM before real work

TRACE = False        # set by test.py to profile
LAST_RESULT = {}     # exec_time_ns etc. for test.py

_CACHED_NC = None


def _build_kernel():
    import concourse.bass as bass
    import concourse.mybir as mybir
    import concourse.tile as tile

    f32 = mybir.dt.float32
    nc = bass.Bass()
    xw_d = nc.dram_tensor("xw", [KT, NKT * COLS], f32, kind="ExternalInput")
    o_d = nc.dram_tensor("o", [NHW, MB], f32, kind="ExternalOutput")

    with tile.TileContext(nc) as tc:
        with (
            tc.tile_pool(name="xwp", bufs=len(CHUNKS)) as xwp,
            tc.tile_pool(name="wu", bufs=1) as wu,
            tc.tile_pool(name="ep", bufs=1) as ep,
            tc.tile_pool(name="pp", bufs=1, space="PSUM") as pp,
            tc.tile_pool(name="pw", bufs=1, space="PSUM") as pw,
        ):
            # --- PE warmup: keep the PE busy (and HAM un-throttled) while the
            # entry preamble and first DMA chunks are in flight.
            warm = wu.tile([KT, 32], f32)
            wps = pw.tile([32, 32], f32)
            nc.vector.memset(warm[:], 0.0)
            for _ in range(WARMUP_MM):
                nc.tensor.matmul(wps[:], warm[:, :32], warm[:], start=True, stop=True)
            # Prewarm ACT tables used by the epilogue.
            wact = wu.tile([1, 1], f32)
            nc.scalar.square(wact[:], warm[:1, :1])
            nc.scalar.add(wact[:], wact[:], 1.0)

            bufs = []
            t0 = 0
            for gi, tpg in enumerate(CHUNKS):
                xwg = xwp.tile([KT, tpg * COLS], f32, tag="xw")
                eng = nc.sync if gi % 2 == 0 else nc.scalar
                eng.dma_start(
                    out=xwg[:], in_=xw_d[:, t0 * COLS:(t0 + tpg) * COLS]
                )
                bufs.append((xwg, t0, tpg))
                t0 += tpg

            # W-half is the stationary operand (80 cols -> cheap LDWEIGHTS,
            # which is the PE bottleneck for fp32: ~2cyc/col at 1.2 GHz);
            # the 128 x columns stream as the moving operand and hide under
            # it. Output comes out transposed: psum[cd, b].
            ps = pp.tile([NHW, MB], f32)
            for xwg, t0, tpg in bufs:
                for j in range(tpg):
                    t = t0 + j
                    nc.tensor.matmul(
                        ps[:],
                        xwg[:, j * COLS + MB:(j + 1) * COLS],
                        xwg[:, j * COLS:j * COLS + MB],
                        start=(t == 0),
                        stop=(t == NKT - 1),
                    )

            # epilogue: s = ps*ALPHA; v = s*|s| / (1 + s^2)
            # ACT computes q2=(ALPHA*ps)^2 and d2=q2+1 in parallel with DVE's
            # s, -s, |s|, s*|s|; DVE finishes with r=1/d2 and v=n*r.
            s = ep.tile([NHW, MB], f32)
            ng = ep.tile([NHW, MB], f32)
            a = ep.tile([NHW, MB], f32)
            n = ep.tile([NHW, MB], f32)
            q2 = ep.tile([NHW, MB], f32)
            d2 = ep.tile([NHW, MB], f32)
            r = ep.tile([NHW, MB], f32)
            v = ep.tile([NHW, MB], f32)
            nc.scalar.activation(q2[:], ps[:], mybir.ActivationFunctionType.Square,
                                 scale=ALPHA)
            nc.scalar.add(d2[:], q2[:], 1.0)
            nc.vector.tensor_scalar_mul(s[:], ps[:], ALPHA)
            nc.vector.tensor_scalar_mul(ng[:], ps[:], -ALPHA)
            nc.vector.tensor_tensor(a[:], s[:], ng[:], mybir.AluOpType.max)
            nc.vector.tensor_mul(n[:], s[:], a[:])
            nc.vector.reciprocal(r[:], d2[:])
            nc.vector.tensor_mul(v[:], n[:], r[:])
            nc.sync.dma_start(out=o_d[:], in_=v[:])
    _split_multi_waits(nc)
    return nc


def _split_multi_waits(nc):
    """TRN2 instructions carry at most one semaphore wait; walrus rejects
    more. Tile's auto-emitted kernel-tail Drain waits on every engine/DMA
    sem. Split extra waits into standalone single-wait EventSemaphore
    instructions placed just before the owner, on the same engine."""
    import concourse.mybir as mybir

    for f in nc.m.functions:
        for blk in f.blocks:
            out = []
            changed = False
            for inst in blk.instructions:
                si = inst.sync_info
                waits = list(si.on_wait) if si and si.on_wait else []
                if len(waits) > 1:
                    changed = True
                    for k, w in enumerate(waits[:-1]):
                        out.append(mybir.InstEventSemaphore(
                            name=f"{inst.name}-sw{k}",
                            engine=inst.engine,
                            ins=[],
                            outs=[],
                            sync_info=mybir.SyncInfo(on_wait=[w], on_update=[]),
                        ))
                    inst.sync_info = mybir.SyncInfo(
                        on_wait=[waits[-1]],
                        on_update=list(si.on_update) if si.on_update else [],
                    )
                out.append(inst)
            if changed:
                blk.instructions = out


def _prep_inputs(x, W):
    """Build the per-core [k, t, (x|w)] interleaved operand arrays."""
    xr = np.ascontiguousarray(x, dtype=np.float32).reshape(BS, K)
    xgs = []
    for g in range(BG):
        xg = xr[g * MB:(g + 1) * MB, :].T.reshape(NKT, KT, MB)  # (t, k, b)
        xgs.append(np.transpose(xg, (1, 0, 2)))                  # (k, t, b)
    Wf = np.ascontiguousarray(
        np.asarray(W, dtype=np.float32)[0].transpose(0, 3, 1, 2)
    ).reshape(K, CD)
    whs = []
    for h in range(NH):
        wh = Wf[:, h * NHW:(h + 1) * NHW].reshape(NKT, KT, NHW)  # (t, k, n)
        whs.append(np.transpose(wh, (1, 0, 2)))                  # (k, t, n)
    maps = []
    for i in range(NCORES):
        g, h = i % BG, i // BG
        xw = np.concatenate([xgs[g], whs[h]], axis=2)            # (k, t, 208)
        maps.append({"xw": np.ascontiguousarray(xw).reshape(KT, NKT * COLS)})
    return maps


def kernel(x, W):
    global _CACHED_NC, LAST_RESULT
    from concourse.bass_utils import run_bass_kernel_spmd

    x = np.asarray(x, dtype=np.float32)
    W = np.asarray(W, dtype=np.float32)
    assert x.shape == (BS, P, E), x.shape
    assert W.shape == (1, P, C, D, E), W.shape

    if _CACHED_NC is None:
        _CACHED_NC = _build_kernel()
    nc = _CACHED_NC

    in_maps = _prep_inputs(x, W)
    res = run_bass_kernel_spmd(nc, in_maps, core_ids=list(range(NCORES)), trace=TRACE)
    LAST_RESULT = {"exec_time_ns": res.exec_time_ns,
                   "mean_exec_time_ns": res.mean_exec_time_ns,
                   "trace": res.instructions_and_trace}

    out = np.empty((BS, CD), dtype=np.float32)
    for i in range(NCORES):
        g, h = i % BG, i // BG
        out[g * MB:(g + 1) * MB, h * NHW:(h + 1) * NHW] = res.results[i]["o"].T
    return out.reshape(BS, C, D, 1)
